# revision 12
# baseline (speedup 1.0000x reference)
"""Trainium2 Bass kernel: MixedScore MultiHeadAttention (fp8 DoubleRow).

Math (per batch b, head h):
  S[r,c]   = (q[b,h,r,:] . k[b,h,c,:]) / 4
  t_m[r,c] = A_m*S + C_m*Q + B_m          (Q = cost_mat[b]; A,C,B have |w2| folded)
  mixed    = sum_m sign(w2_m) * relu(t_m)     (b2 dropped: softmax shift-invariant)
  out      = softmax_c(mixed) @ v

Key ideas vs the fp32r baseline (329 us):
  - S never materialized: A_m*k_c/4 is folded host-side into the mix1
    stationary, so mix1 contracts directly over q's d-dim.
  - All mix matmuls run as fp8e4 DoubleRow (2 K-slices per pass, 0.5
    cycles/row): slice 0 = q-block (q8, ones, q-residual, q8 copy, 1/16
    row), slice 1 = cost-block (cost8, cost-residual).
  - fp8 quantization error is killed by residual rows: moving rows carry
    16*(x - fp8(x)) and the stationary carries the matching /16 weights;
    the stationary's own rounding error W2 = lam*Ak - fp8(lam*Ak) is
    compensated through a second q8 copy. Per-column power-of-2 scales
    lam keep everything in fp8 normal range; mix2 unscales exactly
    (sign * 2^-e is exact in fp8). Emulated end-to-end rel err ~8e-3.
  - relu drains (the vector-engine floor: 33.5M hidden elems/core) are
    split across ACT, DVE and GPSIMD(Pool), two PSUM banks per pass.
  - PV: wexp tiles hold 128 c's on partitions; lhsT = [v | ones] so one
    K=128 fp32r matmul accumulates both out and the softmax denominator.

Layout per core (core = (b, half-of-heads), 8 heads/core):
  qcost[2] (128, NJ, 2, 512) fp8: per j: [q-block | cost-block_j] moving
  pairs; q-block rewritten per head (DMA), cost written once.
  w1k[hh]  (128, NJ, NG, 2, 128) fp8 stationaries (k folded in).
  w2k[hh]  (128, NJ, 4, 2, 32) fp8: sign*2^-e mix2 stationaries.
  hidden   (128, 2, 512) fp8 pair tiles -> mix2 DoubleRow rhs.
  mixed    psum (128, 512) per 2 j-chunks -> exp (ACT) -> PV fp32r.
"""

import os
import sys

import numpy as np
import ml_dtypes

sys.path.insert(0, "/opt/trn_rl_repo")

import concourse.bass as bass  # noqa: E402
import concourse.mybir as mybir  # noqa: E402
from concourse import bacc, tile  # noqa: E402
from concourse.bass_utils import run_bass_kernel_spmd  # noqa: E402

FP = mybir.dt.float32
FPR = mybir.dt.float32r
F8 = mybir.dt.float8e4
F8NP = ml_dtypes.float8_e4m3
DR = mybir.MatmulPerfMode.DoubleRow

B, H, R, C, D, M = 4, 16, 512, 512, 16, 16
HPC = 8  # heads per core
NCORES = 8
NJ = 8  # 64-column chunks per head
NG = 8  # groups (of 8 c's) per chunk
QROWS = 64  # q-block rows: q8(16) ones(1) r16(16) q8(16) inv16(1) zeros(50:64)

AF = mybir.ActivationFunctionType
ALU = mybir.AluOpType

last_results = None  # BassKernelResults of the most recent run (for test.py)

# relu alternates ACT/DVE per 2-bank pair tile (GPSIMD cannot read PSUM);
# ACT also does exp (4/head) and the out copy.


def build_bass():
    nc = bacc.Bacc(None, target_bir_lowering=False, debug=False)

    qblk = nc.declare_dram_parameter("qblk", [QROWS, HPC, NJ, R], F8, isOutput=False)
    cblk = nc.declare_dram_parameter("cblk", [128, NJ, R], F8, isOutput=False)
    w1k = nc.declare_dram_parameter("w1k", [128, HPC, NJ, NG, 2, 128], F8, isOutput=False)
    w2k = nc.declare_dram_parameter("w2k", [128, HPC, NJ, 4, 2, 64], F8, isOutput=False)
    vxx = nc.declare_dram_parameter("vxx", [64, HPC, NJ, D + 1], FPR, isOutput=False)
    outp = nc.declare_dram_parameter("out", [HPC, D + 1, R], FP, isOutput=True)

    with tile.TileContext(nc) as tc:
        with (
            tc.tile_pool(name="const", bufs=1) as constp,
            tc.tile_pool(name="qc", bufs=1) as qcp,
            tc.tile_pool(name="hid", bufs=6) as hidp,
            tc.tile_pool(name="wexp", bufs=3) as wexpp,
            tc.tile_pool(name="osb", bufs=3) as osbp,
            tc.tile_pool(name="ps", bufs=2, space="PSUM") as psp,
            tc.tile_pool(name="pmx", bufs=1, space="PSUM") as pmxp,
            tc.tile_pool(name="pv", bufs=1, space="PSUM") as pvp,
        ):
            w1sb = [
                constp.tile([128, NJ, NG, 2, 128], F8, name=f"w1_{h}", tag=f"w1_{h}")
                for h in range(HPC)
            ]
            w2sb = [
                constp.tile([128, NJ, 4, 2, 64], F8, name=f"w2_{h}", tag=f"w2_{h}")
                for h in range(HPC)
            ]
            vxsb = constp.tile([64, HPC, NJ, D + 1], FPR)
            qc = [
                qcp.tile([128, NJ, 2, R], F8, name=f"qc{t}", tag=f"qc{t}")
                for t in range(2)
            ]

            # cost blocks + zero tails once; q-blocks per head (double-buffered)
            for t in range(2):
                for j in range(NJ):
                    nc.sync.dma_start(out=qc[t][:, j, 1, :], in_=cblk[:, j, :])
                nc.vector.memset(qc[t][QROWS:128, :, 0, :], 0.0)
                nc.sync.dma_start(out=qc[t][0:QROWS, :, 0, :], in_=qblk[:, t])
            nc.sync.dma_start(out=vxsb[:], in_=vxx[:])
            for hh in range(HPC):
                nc.sync.dma_start(out=w1sb[hh][:], in_=w1k[:, hh])
                nc.sync.dma_start(out=w2sb[hh][:], in_=w2k[:, hh])

            for hh in range(HPC):
                qcb = qc[hh % 2]
                pvT = pvp.tile([D + 1, R], FP, name="pvT", tag="pvT")
                for j in range(NJ):
                    if j % 2 == 0:
                        pmx = pmxp.tile([64, 2, R], FP, name="pmx", tag="pmx")
                    mov = qcb[:, j, :, :]
                    hps = []
                    for pr in range(4):
                        ps = psp.tile([128, 2, R], FP, name="ps", tag="ps")
                        for i in range(2):
                            g = 2 * pr + i
                            nc.tensor.matmul(
                                ps[:, i, :],
                                lhsT=w1sb[hh][:, j, g, :, :],
                                rhs=mov,
                                start=True,
                                stop=True,
                                perf_mode=DR,
                            )
                        hp = hidp.tile([128, 2, R], F8, name="hp", tag="hp")
                        if pr % 2 == 0:
                            nc.scalar.activation(hp[:], ps[:], AF.Relu)
                        else:
                            nc.vector.tensor_scalar_max(hp[:], ps[:], 0.0)
                        hps.append(hp)
                    for pr in range(4):
                        nc.tensor.matmul(
                            pmx[:, j % 2, :],
                            lhsT=w2sb[hh][:, j, pr, :, :],
                            rhs=hps[pr][:],
                            start=(pr == 0),
                            stop=(pr == 3),
                            perf_mode=DR,
                        )
                    if j % 2 == 1:
                        wex = wexpp.tile([64, 2, R], FPR, name="wex", tag="wex")
                        nc.scalar.activation(wex[:], pmx[:], AF.Exp)
                        for sl in range(2):
                            nc.tensor.matmul(
                                pvT[:],
                                lhsT=vxsb[:, hh, j - 1 + sl, :],
                                rhs=wex[:, sl, :],
                                start=(j == 1 and sl == 0),
                                stop=(j == 7 and sl == 1),
                            )
                ot = osbp.tile([D + 1, R], FP, name="ot", tag="ot")
                nc.scalar.copy(out=ot[:], in_=pvT[:])
                nc.sync.dma_start(out=outp[hh], in_=ot[:])
                if hh + 2 < HPC:
                    nc.sync.dma_start(
                        out=qc[hh % 2][0:QROWS, :, 0, :], in_=qblk[:, hh + 2]
                    )
    nc.finalize()
    return nc


def _q8(x):
    return np.asarray(x, np.float32).astype(F8NP)


def prepare_in_maps(q, k, v, cost_mat, mix1_weight, mix1_bias, mix2_weight, mix2_bias):
    q = np.asarray(q, np.float32)
    k = np.asarray(k, np.float32)
    v = np.asarray(v, np.float32)
    cost_mat = np.asarray(cost_mat, np.float32)
    w1 = np.asarray(mix1_weight, np.float32)
    b1 = np.asarray(mix1_bias, np.float32)
    w2 = np.asarray(mix2_weight, np.float32)[:, :, 0]

    in_maps = []
    for core in range(NCORES):
        b = core // 2
        h0 = (core % 2) * HPC

        # ---- cost blocks (shared across heads) ----
        Q = cost_mat[b]  # (r, c)
        Q8 = _q8(Q)
        cres = _q8(16.0 * (Q - Q8.astype(np.float32)))
        cblk = np.empty((128, NJ, R), F8NP)
        # partition p<64: cost8 for c=64j+p ; p>=64: residual
        cblk[0:64] = Q8.T.reshape(NJ, 64, R).transpose(1, 0, 2)
        cblk[64:128] = cres.T.reshape(NJ, 64, R).transpose(1, 0, 2)

        # ---- q blocks (per head, duplicated over j) ----
        qh = q[b, h0 : h0 + HPC]  # (HPC, r, d)
        q8 = _q8(qh)
        r16 = _q8(16.0 * (qh - q8.astype(np.float32)))
        qblk = np.zeros((QROWS, HPC, NJ, R), F8NP)
        q8T = q8.transpose(0, 2, 1)  # (HPC, d, r)
        r16T = r16.transpose(0, 2, 1)
        qblk[0:16] = q8T.transpose(1, 0, 2)[:, :, None, :]
        qblk[16] = np.float32(1.0)
        qblk[17:33] = r16T.transpose(1, 0, 2)[:, :, None, :]
        qblk[33:49] = qblk[0:16]
        qblk[49] = np.float32(0.0625)

        # ---- mix1 stationaries with k folded in ----
        aw = np.abs(w2[h0 : h0 + HPC])  # (HPC, M)
        sg = np.sign(w2[h0 : h0 + HPC]).astype(np.float32)
        A = w1[h0 : h0 + HPC, 0] * aw
        Cc = w1[h0 : h0 + HPC, 1] * aw
        Bb = b1[h0 : h0 + HPC] * aw
        kh = k[b, h0 : h0 + HPC]  # (HPC, c, d)
        Ak = np.einsum("hcd,hm->hcmd", kh, A) / 4.0  # (HPC, C, M, D)
        colmax = np.maximum(
            np.abs(Ak).max(-1),
            np.maximum(np.abs(Bb)[:, None, :], np.abs(Cc)[:, None, :]),
        )  # (HPC, C, M)
        e = np.clip(np.round(-np.log2(colmax)), 0, 6)
        lam = (2.0**e).astype(np.float32)
        lAk = lam[..., None] * Ak
        W0 = _q8(lAk)
        W1 = _q8(lAk / 16.0)
        W2 = _q8(lAk - W0.astype(np.float32))
        lB = lam * Bb[:, None, :]
        WB0 = _q8(lB)
        WB1 = _q8(16.0 * (lB - WB0.astype(np.float32)))
        lC = lam * Cc[:, None, :]
        WC0 = _q8(lC)
        WC1 = _q8(lC / 16.0)

        w1k = np.zeros((128, HPC, NJ, NG, 2, 128), F8NP)
        # c = 64j + 8g + c8 ; stationary col = c8*16 + m
        def by_cols(X):  # (HPC, C, M[, D]) -> (HPC, NJ, NG, c8*16+m[, D])
            X = X.reshape((HPC, NJ, NG, 8, M) + X.shape[3:])
            return X.reshape((HPC, NJ, NG, 128) + X.shape[5:])

        W0c, W1c, W2c = by_cols(W0), by_cols(W1), by_cols(W2)  # (..., 128, D)
        w1k[0:16, :, :, :, 0, :] = W0c.transpose(4, 0, 1, 2, 3)
        w1k[16, :, :, :, 0, :] = by_cols(WB0)
        w1k[17:33, :, :, :, 0, :] = W1c.transpose(4, 0, 1, 2, 3)
        w1k[33:49, :, :, :, 0, :] = W2c.transpose(4, 0, 1, 2, 3)
        w1k[49, :, :, :, 0, :] = by_cols(WB1)
        WC0c, WC1c = by_cols(WC0), by_cols(WC1)  # (HPC, NJ, NG, 128)
        c8i = np.arange(8)
        coli = (c8i[:, None] * 16 + np.arange(M)[None, :])  # (8, M)
        for g in range(NG):
            rows = 8 * g + c8i
            # (HPC, NJ, 8c8, M) values at [row, h, j, g, 1, col]
            vals0 = WC0c[:, :, g].reshape(HPC, NJ, 8, M)
            vals1 = WC1c[:, :, g].reshape(HPC, NJ, 8, M)
            for c8 in range(8):
                w1k[rows[c8], :, :, g, 1, coli[c8]] = vals0[:, :, c8].transpose(
                    2, 0, 1
                )
                w1k[64 + rows[c8], :, :, g, 1, coli[c8]] = vals1[:, :, c8].transpose(
                    2, 0, 1
                )

        # ---- mix2 stationaries: sign * 2^-e ----
        s2 = sg[:, None, :] * (2.0 ** (-e))  # (HPC, C, M) at col layout
        s2c = by_cols(s2)  # (HPC, NJ, NG, 128=c8*16+m)
        w2k = np.zeros((128, HPC, NJ, 4, 2, 64), F8NP)
        for pr in range(4):
            for i in range(2):
                g = 2 * pr + i
                for c8 in range(8):
                    # partition p = c8*16+m, col = c_local = 8g + c8
                    w2k[coli[c8], :, :, pr, i, 8 * g + c8] = s2c[
                        :, :, g, coli[c8]
                    ].transpose(2, 0, 1)

        # ---- v with ones column ----
        vh = v[b, h0 : h0 + HPC]  # (HPC, C, D)
        vxa = np.zeros((64, HPC, NJ, D + 1), np.float32)
        vxa[:, :, :, :D] = vh.reshape(HPC, NJ, 64, D).transpose(2, 0, 1, 3)
        vxa[:, :, :, D] = 1.0

        in_maps.append(dict(qblk=qblk, cblk=cblk, w1k=w1k, w2k=w2k, vxx=vxa))
    return in_maps


def assemble(results):
    full = np.empty((B, R, H * D), np.float32)
    for core in range(NCORES):
        b = core // 2
        c0 = (core % 2) * HPC * D
        o = results[core]["out"]  # (HPC, D+1, R); row D is the softmax denom
        o = o[:, :D, :] / o[:, D : D + 1, :]
        full[b, :, c0 : c0 + HPC * D] = o.transpose(2, 0, 1).reshape(R, HPC * D)
    return full


_nc_cache = None


def _install_ntff_hook():
    """The agent image's antenv lacks axon_hooks; recreate it and register
    the ctypes NTFF profiling hook so trace=True yields exec times."""
    import types

    try:
        import antenv

        try:
            import antenv.axon_hooks  # noqa: F401

            return
        except ImportError:
            pass
        mod = types.ModuleType("antenv.axon_hooks")
        mod._hook = None
        mod.set_axon_ntff_profile_hook = lambda h: setattr(mod, "_hook", h)
        mod.get_axon_ntff_profile_hook = lambda: mod._hook
        sys.modules["antenv.axon_hooks"] = mod
        antenv.axon_hooks = mod
        from trn_agent_boot.trn_boot import _ntff_profile_via_ctypes

        mod._hook = _ntff_profile_via_ctypes("/opt/axon/libaxon_pjrt.so")
    except Exception as e:  # profiling is best-effort
        print(f"ntff hook install failed: {e}", file=sys.stderr)


def kernel(**inputs) -> np.ndarray:
    global _nc_cache, last_results
    if _nc_cache is None:
        _nc_cache = build_bass()
    in_maps = prepare_in_maps(**inputs)
    trace = bool(int(os.environ.get("KERNEL_TRACE", "0")))
    if trace:
        _install_ntff_hook()
        import concourse.bass_utils as bu

        bu.upload_artifacts = lambda tmpdir: f"local:{tmpdir}"
    res = run_bass_kernel_spmd(_nc_cache, in_maps, list(range(NCORES)), trace=trace)
    last_results = res
    return assemble(res.results)
